# revision 9
# baseline (speedup 1.0000x reference)
"""Trainium2 Bass kernel for nn_Net_21852793602541 (gnn_message_passing).

The reference net's output depends only on a tiny dependency cone of the
message-passing graph: the final hidden layer reads the wave-2 snapshot of
neuron activations, so only neurons feeding neuron 255 through channels whose
source was already processed matter.  For the fixed graph that is a 3-conv
chain (x -> n0 -> n172 -> n215), one 784->200 FC block, a 200->10 FC and
log_softmax.  The cone is recomputed at runtime from the src/tgt inputs.

Per-core mapping (data-parallel over batch, 16 images/core on 8 cores):
  * 5x5 conv == one PE accumulation group: contraction K = (dy, slot-row)
    with a banded-Toeplitz stationary (fp16).  Activations live in a single
    [128, 512] stack whose free axis is (ypad, b): the four dy<=3 groups are
    y-shifted replicas on partition groups, and the dy=4 tap is realised as a
    column-offset read of group 0 - no fifth replica, no 32-partition tail.
  * Fanout of a conv result into the next stack reads PSUM directly from both
    ACT (one group) and DVE (three groups) in parallel; bias+relu is fused
    into every write.
  * fc1 is computed transposed (hidden on partitions) via 14 small matmuls so
    no PSUM->SBUF copy or PE transpose is needed; fc2 takes the hidden tiles
    as stationary with fc2_b folded in through an all-ones row.
  * log_softmax drops the max-subtraction (logits are small), uses Exp with
    fused accumulation and Ln; a post-compile pass rewrites all activation
    table loads into a single preamble load of the set containing both exp
    and ln, so no table switch lands on the critical path.
  * PE clock-gate warmup: back-to-back dummy matmuls start as soon as the
    engines come up, so the HAM un-throttles (1.2 -> 2.4 GHz) right as the
    input DMAs complete and the real conv chain begins.
"""

import numpy as np

import concourse.bass as bass
import concourse.tile as tile
from concourse import bacc, mybir
from concourse.bass_utils import run_bass_kernel_spmd

# The axon NTFF profile hook normally lives in antenv.axon_hooks, which this
# image lacks.  Shim it from the boot module's ctypes implementation so
# BASS_TRACE=1 profiling works; degrade silently if unavailable.
try:
    import antenv.axon_hooks  # noqa: F401
except ImportError:
    try:
        import sys as _sys
        import types as _types

        from trn_agent_boot.trn_boot import _ntff_profile_via_ctypes

        _hook = _ntff_profile_via_ctypes('/opt/axon/libaxon_pjrt.so')
        _mod = _types.ModuleType('antenv.axon_hooks')
        _mod.get_axon_ntff_profile_hook = lambda: _hook
        _mod.set_axon_ntff_profile_hook = lambda h: None
        _sys.modules['antenv.axon_hooks'] = _mod
    except Exception:
        pass

F32 = mybir.dt.float32
F16 = mybir.dt.float16
AF = mybir.ActivationFunctionType
ALU = mybir.AluOpType
N_NEURONS = 256
N_CORES = 8
B_TOTAL = 128
B = B_TOTAL // N_CORES  # 16 images per core
HW = 28
FC_HID = 200
N_CLS = 10
NWARM = 13  # dummy matmuls to trip the PE clock gate during the DMA wait

LAST_RESULT = None  # BassKernelResults of the most recent run (for profiling)


# ---------------------------------------------------------------- schedule
def _schedule(src, tgt):
    n = N_NEURONS
    in_lists = [src[np.where(tgt == i)[0]].astype(np.int64).tolist() for i in range(n)]
    waves = []
    processed = np.zeros(n, bool)
    frontier = [0]
    while True:
        waves.append(list(frontier))
        processed[frontier] = True
        if processed[n - 1]:
            break
        nxt = set()
        for v in frontier:
            for m in tgt[src == v]:
                if not processed[m]:
                    nxt.add(int(m))
        frontier = sorted(nxt)
        assert frontier, "last neuron unreachable"
    return in_lists, waves


def _cone(src, tgt):
    """Returns (steps, fc_live).

    steps: ordered list of (node, [(srckey, channel), ...]) where srckey is
      'x' for the image input or an int neuron id computed in an earlier step.
    fc_live: [(channel_of_255, src_node), ...] live channels of the readout.
    """
    n = N_NEURONS
    in_lists, waves = _schedule(src, tgt)
    wave_of = {}
    for wi, w in enumerate(waves):
        for v in w:
            if v not in wave_of:
                wave_of[v] = wi
    BIG = 1 << 30
    w255 = wave_of[n - 1]
    fc_live = [(c, int(s)) for c, s in enumerate(in_lists[n - 1])
               if wave_of.get(int(s), BIG) < w255]

    live = {}
    stack = [s for _, s in fc_live]
    seen = set()
    while stack:
        v = stack.pop()
        if v in seen:
            continue
        seen.add(v)
        if v == 0:
            live[0] = [('x', 0)]
            continue
        chans = [(int(s), c) for c, s in enumerate(in_lists[v])
                 if wave_of.get(int(s), BIG) < wave_of[v]]
        assert chans, f"cone node {v} has no live channels"
        live[v] = [(s, c) for s, c in chans]
        stack += [s for s, _ in chans]

    steps = sorted(live.items(), key=lambda kv: wave_of[kv[0]])
    return steps, fc_live


# ---------------------------------------------------------- host-side packing
def _toeplitz(w):
    """w [5,5] -> [160, 28] banded matrix over K=(dy, slot-row).

    Slot row r of each 32-row group holds padded-image column (r+2) mod 32,
    so the activation value at x lands at row x (32-aligned writes; wrapped
    rows 28..31 hold the zero x-padding)."""
    T = np.zeros((160, HW), np.float32)
    for dy in range(5):
        for dx in range(5):
            for xc in range(HW):
                T[dy * 32 + (xc + dx - 2) % 32, xc] = w[dy, dx]
    return T


def _xstack(xb):
    """xb [B,28,28] -> [128, 544] fp16 stack, free axis = (ypad, b).

    Group g (dy=g<=3), slot row r, column p*16+b holds
    xpad[b, p+g-2, (r+2) % 32] (zero when the y index is out of range).
    The dy=4 tap is read from group 0 at a +96-column offset on device
    (columns [512, 544) stay zero - the overhang rows are y-padding)."""
    xpad = np.zeros((B, 32, 32), np.float32)
    xpad[:, 2:30, 2:30] = xb
    st = np.zeros((4, 32, 34, B), np.float32)  # g, slot-x, ypad, b
    for g in range(4):
        lo, hi = max(0, 2 - g), min(32, 34 - g)
        st[g, :, lo:hi, :] = xpad[:, lo + g - 2:hi + g - 2, :].transpose(2, 1, 0)
    st = np.roll(st, -2, axis=1)  # slot row r holds padded col (r+2) % 32
    return st.reshape(128, 34 * B).astype(np.float16)


class _Layout:
    def __init__(self):
        self.n = 0

    def alloc(self, w):
        c0 = self.n
        self.n += w
        return c0


def _pack(steps, fc_live, conv_w, conv_b, fc1_w, fc1_b, fc2_w, fc2_b):
    """Builds consts (f32), mainh head block (fp16), f1w (fp16)."""
    slots = {}
    lay32 = _Layout()
    lay16 = _Layout()
    for v, chans in steps:
        for j, _ in enumerate(chans):
            slots[('toepA', v, j)] = lay16.alloc(HW)
            slots[('toepB', v, j)] = lay16.alloc(HW)
        slots[('cb', v)] = lay32.alloc(1)
    slots['fc1bA'] = lay32.alloc(1)
    slots['fc1bB'] = lay32.alloc(1)
    slots['w2A'] = lay16.alloc(N_CLS)
    slots['w2B'] = lay16.alloc(N_CLS)
    head_cols = lay16.n
    slots['xs'] = lay16.alloc(544)

    C = np.zeros((128, lay32.n), np.float32)
    TH = np.zeros((128, head_cols), np.float16)
    for v, chans in steps:
        for j, (skey, ch) in enumerate(chans):
            T = _toeplitz(conv_w[v, 0, ch])
            TH[:, slots[('toepA', v, j)]:slots[('toepA', v, j)] + HW] = T[:128]
            TH[:32, slots[('toepB', v, j)]:slots[('toepB', v, j)] + HW] = T[128:]
        C[:HW, slots[('cb', v)]] = conv_b[v]
    C[:128, slots['fc1bA']] = fc1_b[:128]
    C[:FC_HID - 128, slots['fc1bB']] = fc1_b[128:]
    w2t = fc2_w.T  # [200, 10]
    TH[:128, slots['w2A']:slots['w2A'] + N_CLS] = w2t[:128]
    TH[:FC_HID - 128, slots['w2B']:slots['w2B'] + N_CLS] = w2t[128:]
    TH[FC_HID - 128, slots['w2B']:slots['w2B'] + N_CLS] = fc2_b  # ones-row bias

    # fc1 transposed: stationary chunks [128, 200] per (live channel, ysub).
    # Partition = yg*32 + x, column (k*7+sj)*200 + j = fc1_w[j, pixel].
    n_fc = len(fc_live)
    f1p = np.zeros((128, 1400 * n_fc), np.float16)
    for k, (c, s) in enumerate(fc_live):
        blk = fc1_w[:, c * 784:(c + 1) * 784].reshape(FC_HID, 4, 7, HW)  # j,g,sj,x
        arr = blk.transpose(1, 3, 2, 0)  # g, x, sj, j
        arr = np.pad(arr, ((0, 0), (0, 4), (0, 0), (0, 0)))  # x -> 32
        f1p[:, k * 1400:(k + 1) * 1400] = arr.reshape(128, 1400)
    return C, TH, f1p, slots


# -------------------------------------------------- activation-table surgery
def _fuse_act_tables(nc):
    """Rewrite the compiler-inserted per-function activation table loads into
    one preamble load of a set containing every function the kernel uses.

    The greedy chooser picks the first set containing each function (exp ->
    set 0, ln -> set 5) and reloads on every switch, putting a ~1.5us
    ACT_TABLE_LOAD between Exp and Ln on the critical path.  A single set
    (natural_log_exp_and_others) contains exp, ln, relu, identity and copy,
    so one load before the first activation covers the whole kernel."""
    from concourse.hw_specs import get_activation_tables
    tables = list(get_activation_tables(nc.m.arch).items())
    used = set()
    loads = []
    for b in nc.main_func.blocks:
        for i in b.instructions:
            if isinstance(i, mybir.InstActivation):
                used.add(i.func)
            elif isinstance(i, mybir.InstLoadActFuncSet):
                loads.append((b, i))
    if len(loads) <= 1:
        return
    pick = None
    for idx, (_, fns) in enumerate(tables):
        if used <= fns:
            pick = idx
            break
    if pick is None:
        return
    first = True
    for b, i in loads:
        if first:
            i.act_func_set_id = pick
            first = False
            continue
        si = i.sync_info
        if si is not None and (si.on_wait or si.on_update):
            continue  # carries sync - leave it (redundant but harmless)
        b.instructions.remove(i)


# ---------------------------------------------------------- device program
def _build(steps, fc_live, ncols32, ncols16, nfc):
    nc = bacc.Bacc("TRN2", target_bir_lowering=False)
    consts_d = nc.dram_tensor("consts", [128, ncols32], F32, kind="ExternalInput")
    mainh_d = nc.dram_tensor("mainh", [128, ncols16], F16, kind="ExternalInput")
    f1w_d = nc.dram_tensor("f1w", [128, 1400 * nfc], F16, kind="ExternalInput")
    out_d = nc.dram_tensor("out", [B, N_CLS], F32, kind="ExternalOutput")

    feeds_conv = set()
    for v, chans in steps:
        for skey, _ in chans:
            if skey != 'x':
                feeds_conv.add(skey)
    fc_srcs = [s for _, s in fc_live]
    SL = _SLOTS
    H2 = FC_HID - 128  # 72

    with tile.TileContext(nc) as tc:
        with (
            tc.tile_pool(name="persist", bufs=1) as pool,
            tc.tile_pool(name="cpsum", bufs=2, space="PSUM") as cpp,
            tc.tile_pool(name="fpsum", bufs=1, space="PSUM") as fpp,
        ):
            consts = pool.tile([128, ncols32], F32, tag="consts")
            mainh = pool.tile([128, ncols16], F16, tag="mainh")
            f1w = pool.tile([128, 1400 * nfc], F16, tag="f1w")

            # PE clock-gate warmup.  The dummy matmuls deliberately read an
            # UNINITIALISED tile: with no input dependency they start the
            # moment the Tensor engine finishes its preamble (~5.2us), so the
            # HAM un-throttles (1.2 -> 2.4 GHz) at ~8.6us - before the first
            # conv.  Garbage fp16 (even NaN/Inf) is harmless: the PSUM target
            # is never read and every real accumulation starts with
            # start=True.  NWARM spans the gap until the mainh DMA lands.
            dmy = pool.tile([1, 512], F16, tag="dmy")
            warmps = fpp.tile([1, 512], F32, bufs=1)
            for _ in range(NWARM):
                nc.tensor.matmul(warmps[:], dmy[:1, 0:1], dmy[:],
                                 start=True, stop=True)
            # the allocator requires a writer for dmy; this trailing memset
            # (idle GpSimd queue) WAR-waits on the warmups instead of gating
            # them, so it costs nothing
            nc.gpsimd.memset(dmy[:], 1.0)

            # All input DMAs on the sync queue (one hardware ring keeps the
            # semaphore-reset epilogue small), mainh first - it gates conv0.
            nc.sync.dma_start(mainh[:], mainh_d[:])
            nc.sync.dma_start(consts[:], consts_d[:])
            nc.sync.dma_start(f1w[:], f1w_d[:])

            # Trigger the single activation-table load (rewritten to the
            # exp+ln set by _fuse_act_tables) off the critical path.
            swu = pool.tile([1, 2], F32, tag="swu")
            nc.vector.memset(swu[:], 1.0)
            nc.scalar.activation(swu[:, 0:1], swu[:, 0:1], AF.Exp)

            # Activation stacks (fp16, zero borders double as y-padding) and
            # the fc readout stack; h2e's extra row of ones folds fc2_b in.
            stacks = {}
            for v in sorted(feeds_conv):
                a = pool.tile([128, 544], F16, name=f"st_{v}", tag=f"st_{v}")
                nc.vector.memset(a[:], 0.0)
                stacks[v] = a
            fcstacks = {}
            for sv in sorted(set(fc_srcs)):
                t = pool.tile([128, 112 * 1], F16, name=f"fcst_{sv}",
                              tag=f"fcst_{sv}")
                nc.vector.memset(t[:], 0.0)
                fcstacks[sv] = t
            h1 = pool.tile([128, B], F16, tag="h1")
            h2e = pool.tile([H2 + 1, B], F16, tag="h2e")
            nc.vector.memset(h2e[:], 1.0)

            def movA(key):
                src = mainh[:, SL['xs'] + 32:SL['xs'] + 480] if key == 'x' \
                    else stacks[key][:, 32:480]
                return src

            def movB(key):
                src = mainh[0:32, SL['xs'] + 96:SL['xs'] + 544] if key == 'x' \
                    else stacks[key][0:32, 96:544]
                return src

            # --- conv chain ---
            # Tile hazard tracking is partition-blind (column-overlap on the
            # same tile serializes), so the stack fanout is one ACT bias+relu
            # into group 2 followed by three cheap 16-bit DVE copies; fc-only
            # nodes pipeline quarter matmul pairs against the quarter writes.
            for v, chans in steps:
                cb0 = SL[('cb', v)]
                bias = consts[:HW, cb0:cb0 + 1]
                nch = len(chans)
                fc_only = v in fcstacks and v not in feeds_conv

                if fc_only:
                    fst = fcstacks[v]
                    for g in range(4):
                        psq = cpp.tile([HW, 112], F32, tag="convq", bufs=2,
                                       name=f"psq{v}_{g}")
                        for j, (skey, ch) in enumerate(chans):
                            a0 = SL[('toepA', v, j)]
                            b0 = SL[('toepB', v, j)]
                            cA = g * 112
                            nc.tensor.matmul(
                                psq[:], mainh[:, a0:a0 + HW],
                                movA(skey)[:, cA:cA + 112],
                                start=(j == 0), stop=False)
                            nc.tensor.matmul(
                                psq[:], mainh[:32, b0:b0 + HW],
                                movB(skey)[:, cA:cA + 112],
                                start=False, stop=(j == nch - 1))
                        dst = fst[g * 32:g * 32 + HW, :]
                        if g % 2 == 0:
                            nc.scalar.activation(dst, psq[:], AF.Relu,
                                                 bias=bias, scale=1.0)
                        else:
                            nc.vector.tensor_scalar(dst, psq[:], bias, 0.0,
                                                    ALU.add, ALU.max)
                    continue

                ps = cpp.tile([HW, 448], F32, tag="convps", name=f"ps{v}")
                for j, (skey, ch) in enumerate(chans):
                    a0 = SL[('toepA', v, j)]
                    b0 = SL[('toepB', v, j)]
                    nc.tensor.matmul(ps[:], mainh[:, a0:a0 + HW], movA(skey),
                                     start=(j == 0), stop=False)
                    nc.tensor.matmul(ps[:], mainh[:32, b0:b0 + HW], movB(skey),
                                     start=False, stop=(j == nch - 1))
                # insurance dummies keep the PE busy through the fanout gap so
                # the HAM clock gate flips (and stays) at 2.4 GHz
                for _ in range(2):
                    nc.tensor.matmul(warmps[:], dmy[:1, 0:1], dmy[:],
                                     start=True, stop=True)

                if v in feeds_conv:
                    st = stacks[v]
                    g2 = st[64:64 + HW, 32:480]
                    nc.scalar.activation(g2, ps[:], AF.Relu, bias=bias,
                                         scale=1.0)
                    for g in (0, 1, 3):
                        c0 = (4 - g) * 16
                        nc.vector.tensor_copy(
                            st[g * 32:g * 32 + HW, c0:c0 + 448], g2)
                if v in fcstacks:
                    fst = fcstacks[v]
                    for g in range(4):
                        dst = fst[g * 32:g * 32 + HW, :]
                        srcp = ps[:, g * 112:(g + 1) * 112]
                        if g % 2 == 0:
                            nc.scalar.activation(dst, srcp, AF.Relu,
                                                 bias=bias, scale=1.0)
                        else:
                            nc.vector.tensor_scalar(dst, srcp, bias, 0.0,
                                                    ALU.add, ALU.max)

            # --- fc1 transposed: hidden units on partitions ---
            p1a = fpp.tile([128, B], F32, tag="p1a", bufs=1)
            p1b = fpp.tile([H2, B], F32, tag="p1b", bufs=1)
            nmm = nfc * 7
            for k in range(nfc):
                fst = fcstacks[fc_live[k][1]]
                for sj in range(7):
                    i = k * 7 + sj
                    mov = fst[:, sj * 16:(sj + 1) * 16]
                    c0 = i * 200
                    nc.tensor.matmul(p1a[:], f1w[:, c0:c0 + 128], mov,
                                     start=(i == 0), stop=(i == nmm - 1))
                    nc.tensor.matmul(p1b[:], f1w[:, c0 + 128:c0 + 200], mov,
                                     start=(i == 0), stop=(i == nmm - 1))
            nc.scalar.activation(h1[:], p1a[:], AF.Relu,
                                 bias=consts[:128, SL['fc1bA']:SL['fc1bA'] + 1],
                                 scale=1.0)
            nc.vector.tensor_scalar(h2e[0:H2, :], p1b[:],
                                    consts[:H2, SL['fc1bB']:SL['fc1bB'] + 1],
                                    0.0, ALU.add, ALU.max)

            # --- fc2 + log_softmax (no max-subtraction; logits are small) ---
            pst = fpp.tile([B, N_CLS], F32, tag="pst", bufs=1)
            nc.tensor.matmul(pst[:], h1[:], mainh[:, SL['w2A']:SL['w2A'] + N_CLS],
                             start=True, stop=False)
            nc.tensor.matmul(pst[:], h2e[:],
                             mainh[:H2 + 1, SL['w2B']:SL['w2B'] + N_CLS],
                             start=False, stop=True)
            ex = pool.tile([B, N_CLS], F32, tag="ex")
            sm = pool.tile([B, 1], F32, tag="sm")
            nc.scalar.activation(ex[:], pst[:], AF.Exp, bias=0.0, scale=1.0,
                                 accum_out=sm[:])
            lse = pool.tile([B, 1], F32, tag="lse")
            nc.scalar.activation(lse[:], sm[:], AF.Ln, bias=0.0, scale=1.0)
            res = pool.tile([B, N_CLS], F32, tag="res")
            nc.vector.tensor_scalar(res[:], pst[:], lse[:], None, ALU.subtract)
            nc.sync.dma_start(out_d[:], res[:])
    nc.compile()
    _fuse_act_tables(nc)
    return nc


_SLOTS = None
_PROG_CACHE = {}


def kernel(x, src, tgt, conv_w, conv_b, fc1_w, fc1_b, fc2_w, fc2_b):
    global _SLOTS, LAST_RESULT
    x = np.asarray(x, np.float32)
    src = np.asarray(src, np.int32)
    tgt = np.asarray(tgt, np.int32)
    conv_w = np.asarray(conv_w, np.float32)
    conv_b = np.asarray(conv_b, np.float32)
    fc1_w = np.asarray(fc1_w, np.float32)
    fc1_b = np.asarray(fc1_b, np.float32)
    fc2_w = np.asarray(fc2_w, np.float32)
    fc2_b = np.asarray(fc2_b, np.float32)

    steps, fc_live = _cone(src, tgt)
    C, TH, f1p, slots = _pack(steps, fc_live, conv_w, conv_b,
                              fc1_w, fc1_b, fc2_w, fc2_b)
    _SLOTS = slots
    ncols16 = TH.shape[1] + 544

    key = (tuple((v, tuple(ch)) for v, ch in steps), tuple(fc_live),
           C.shape[1], ncols16)
    if key not in _PROG_CACHE:
        _PROG_CACHE[key] = _build(steps, fc_live, C.shape[1], ncols16,
                                  len(fc_live))
    nc = _PROG_CACHE[key]

    xs_all = x[:, 0]  # [128, 28, 28]
    in_maps = []
    for c in range(N_CORES):
        xs = _xstack(xs_all[c * B:(c + 1) * B])
        mainh = np.concatenate([TH, xs], axis=1)
        in_maps.append({"consts": C, "mainh": mainh, "f1w": f1p})

    LAST_RESULT = run_bass_kernel_spmd(nc, in_maps, list(range(N_CORES)))
    out = np.concatenate([r["out"] for r in LAST_RESULT.results], axis=0)
    return out.astype(np.float32)


# revision 13
# speedup vs baseline: 1.0434x; 1.0434x over previous
"""Trainium2 Bass kernel for nn_Net_21852793602541 (gnn_message_passing).

The reference net's output depends only on a tiny dependency cone of the
message-passing graph: the final hidden layer reads the wave-2 snapshot of
neuron activations, so only neurons feeding neuron 255 through channels whose
source was already processed matter.  For the fixed graph that is a 3-conv
chain (x -> n0 -> n172 -> n215), one 784->200 FC block, a 200->10 FC and
log_softmax.  The cone is recomputed at runtime from the src/tgt inputs.

Per-core mapping (data-parallel over batch, 16 images/core on 8 cores):
  * 5x5 conv == one PE accumulation group: contraction K = (dy, slot-row)
    with a banded-Toeplitz stationary.  Activations live in a single
    [128, 544] stack whose free axis is (ypad, b): the four dy<=3 groups are
    y-shifted replicas on partition groups, and the dy=4 tap is realised as a
    column-offset read of group 0 - no fifth replica, no 32-partition tail.
  * Tile hazard tracking is partition-blind and full-tile for writes, so the
    stack fanout is one ACT bias+relu from PSUM into group 2 followed by
    three cheap 16-bit DVE copies; fc-only nodes pipeline quarter matmul
    pairs against alternating ACT/DVE quarter writes.
  * The input image and conv0's toeplitz ride a single fp8e4 tensor (half
    the DMA bytes of fp16; rel err ~5e-4), issued first on the sync queue -
    it alone gates conv0.  Later stacks are fp16.
  * fc1 is computed transposed (hidden on partitions) via 14 small matmuls so
    no PSUM->SBUF copy or PE transpose is needed; fc2 takes the hidden tiles
    as stationary with fc2_b folded in through an all-ones row.
  * log_softmax drops the max-subtraction (logits are small), uses Exp with
    fused accumulation and Ln; a post-compile pass rewrites all activation
    table loads into a single preamble load of the set containing both exp
    and ln, so no table switch lands on the critical path.
  * No PE warmup: the HAM clock gate on this part never un-throttles
    (verified over 5.7us of continuous matmuls), so everything runs at
    1.2 GHz and dummy matmuls would only delay the input DMAs.
"""

import numpy as np

import concourse.bass as bass
import concourse.tile as tile
from concourse import bacc, mybir
from concourse.bass_utils import run_bass_kernel_spmd

# The axon NTFF profile hook normally lives in antenv.axon_hooks, which this
# image lacks.  Shim it from the boot module's ctypes implementation so
# BASS_TRACE=1 profiling works; degrade silently if unavailable.
try:
    import antenv.axon_hooks  # noqa: F401
except ImportError:
    try:
        import sys as _sys
        import types as _types

        from trn_agent_boot.trn_boot import _ntff_profile_via_ctypes

        _hook = _ntff_profile_via_ctypes('/opt/axon/libaxon_pjrt.so')
        _mod = _types.ModuleType('antenv.axon_hooks')
        _mod.get_axon_ntff_profile_hook = lambda: _hook
        _mod.set_axon_ntff_profile_hook = lambda h: None
        _sys.modules['antenv.axon_hooks'] = _mod
    except Exception:
        pass

F32 = mybir.dt.float32
F16 = mybir.dt.float16
F8 = mybir.dt.float8e4
AF = mybir.ActivationFunctionType
ALU = mybir.AluOpType
N_NEURONS = 256
N_CORES = 8
B_TOTAL = 128
B = B_TOTAL // N_CORES  # 16 images per core
HW = 28
FC_HID = 200
N_CLS = 10

LAST_RESULT = None  # BassKernelResults of the most recent run (for profiling)


# ---------------------------------------------------------------- schedule
def _schedule(src, tgt):
    n = N_NEURONS
    in_lists = [src[np.where(tgt == i)[0]].astype(np.int64).tolist() for i in range(n)]
    waves = []
    processed = np.zeros(n, bool)
    frontier = [0]
    while True:
        waves.append(list(frontier))
        processed[frontier] = True
        if processed[n - 1]:
            break
        nxt = set()
        for v in frontier:
            for m in tgt[src == v]:
                if not processed[m]:
                    nxt.add(int(m))
        frontier = sorted(nxt)
        assert frontier, "last neuron unreachable"
    return in_lists, waves


def _cone(src, tgt):
    """Returns (steps, fc_live).

    steps: ordered list of (node, [(srckey, channel), ...]) where srckey is
      'x' for the image input or an int neuron id computed in an earlier step.
    fc_live: [(channel_of_255, src_node), ...] live channels of the readout.
    """
    n = N_NEURONS
    in_lists, waves = _schedule(src, tgt)
    wave_of = {}
    for wi, w in enumerate(waves):
        for v in w:
            if v not in wave_of:
                wave_of[v] = wi
    BIG = 1 << 30
    w255 = wave_of[n - 1]
    fc_live = [(c, int(s)) for c, s in enumerate(in_lists[n - 1])
               if wave_of.get(int(s), BIG) < w255]

    live = {}
    stack = [s for _, s in fc_live]
    seen = set()
    while stack:
        v = stack.pop()
        if v in seen:
            continue
        seen.add(v)
        if v == 0:
            live[0] = [('x', 0)]
            continue
        chans = [(int(s), c) for c, s in enumerate(in_lists[v])
                 if wave_of.get(int(s), BIG) < wave_of[v]]
        assert chans, f"cone node {v} has no live channels"
        live[v] = [(s, c) for s, c in chans]
        stack += [s for s, _ in chans]

    steps = sorted(live.items(), key=lambda kv: wave_of[kv[0]])
    return steps, fc_live


# ---------------------------------------------------------- host-side packing
def _toeplitz(w):
    """w [5,5] -> [160, 28] banded matrix over K=(dy, slot-row).

    Slot row r of each 32-row group holds padded-image column (r+2) mod 32,
    so the activation value at x lands at row x (32-aligned writes; wrapped
    rows 28..31 hold the zero x-padding)."""
    T = np.zeros((160, HW), np.float32)
    for dy in range(5):
        for dx in range(5):
            for xc in range(HW):
                T[dy * 32 + (xc + dx - 2) % 32, xc] = w[dy, dx]
    return T


def _xstack(xb):
    """xb [B,28,28] -> [128, 544] stack (fp32; caller casts), free axis = (ypad, b).

    Group g (dy=g<=3), slot row r, column p*16+b holds
    xpad[b, p+g-2, (r+2) % 32] (zero when the y index is out of range).
    The dy=4 tap is read from group 0 at a +96-column offset on device
    (columns [512, 544) stay zero - the overhang rows are y-padding)."""
    xpad = np.zeros((B, 32, 32), np.float32)
    xpad[:, 2:30, 2:30] = xb
    st = np.zeros((4, 32, 34, B), np.float32)  # g, slot-x, ypad, b
    for g in range(4):
        lo, hi = max(0, 2 - g), min(32, 34 - g)
        st[g, :, lo:hi, :] = xpad[:, lo + g - 2:hi + g - 2, :].transpose(2, 1, 0)
    st = np.roll(st, -2, axis=1)  # slot row r holds padded col (r+2) % 32
    return st.reshape(128, 34 * B)


class _Layout:
    def __init__(self):
        self.n = 0

    def alloc(self, w):
        c0 = self.n
        self.n += w
        return c0


def _pack(steps, fc_live, conv_w, conv_b, fc1_w, fc1_b, fc2_w, fc2_b):
    """Builds consts (f32), mainh head block (fp16), f1w (fp16)."""
    slots = {}
    lay32 = _Layout()
    lay16 = _Layout()
    for v, chans in steps:
        for j, _ in enumerate(chans):
            slots[('toepA', v, j)] = lay16.alloc(HW)
            slots[('toepB', v, j)] = lay16.alloc(HW)
        slots[('cb', v)] = lay32.alloc(1)
    slots['fc1bA'] = lay32.alloc(1)
    slots['fc1bB'] = lay32.alloc(1)
    slots['w2A'] = lay16.alloc(N_CLS)
    slots['w2B'] = lay16.alloc(N_CLS)
    head_cols = lay16.n

    # x-sourced channels run entirely in fp8e4 (input image + conv0 toeplitz
    # share the xs tensor - one DMA, half the bytes, rel err ~4e-4)
    lay8 = _Layout()
    slots['xs'] = lay8.alloc(544)
    for v, chans in steps:
        for j, (skey, ch) in enumerate(chans):
            if skey == 'x':
                slots[('xtoepA', v, j)] = lay8.alloc(HW)
                slots[('xtoepB', v, j)] = lay8.alloc(HW)
    xs_cols = lay8.n

    C = np.zeros((128, lay32.n), np.float32)
    TH = np.zeros((128, head_cols), np.float16)
    X8 = np.zeros((128, xs_cols), np.float32)  # cast to fp8 by caller
    for v, chans in steps:
        for j, (skey, ch) in enumerate(chans):
            T = _toeplitz(conv_w[v, 0, ch])
            TH[:, slots[('toepA', v, j)]:slots[('toepA', v, j)] + HW] = T[:128]
            TH[:32, slots[('toepB', v, j)]:slots[('toepB', v, j)] + HW] = T[128:]
            if skey == 'x':
                a8 = slots[('xtoepA', v, j)]
                b8 = slots[('xtoepB', v, j)]
                X8[:, a8:a8 + HW] = T[:128]
                X8[:32, b8:b8 + HW] = T[128:]
        C[:HW, slots[('cb', v)]] = conv_b[v]
    C[:128, slots['fc1bA']] = fc1_b[:128]
    C[:FC_HID - 128, slots['fc1bB']] = fc1_b[128:]
    w2t = fc2_w.T  # [200, 10]
    TH[:128, slots['w2A']:slots['w2A'] + N_CLS] = w2t[:128]
    TH[:FC_HID - 128, slots['w2B']:slots['w2B'] + N_CLS] = w2t[128:]
    TH[FC_HID - 128, slots['w2B']:slots['w2B'] + N_CLS] = fc2_b  # ones-row bias

    # fc1 transposed: stationary chunks [128, 200] per (live channel, ysub).
    # Partition = yg*32 + x, column (k*7+sj)*200 + j = fc1_w[j, pixel].
    n_fc = len(fc_live)
    f1p = np.zeros((128, 1400 * n_fc), np.float16)
    for k, (c, s) in enumerate(fc_live):
        blk = fc1_w[:, c * 784:(c + 1) * 784].reshape(FC_HID, 4, 7, HW)  # j,g,sj,x
        arr = blk.transpose(1, 3, 2, 0)  # g, x, sj, j
        arr = np.pad(arr, ((0, 0), (0, 4), (0, 0), (0, 0)))  # x -> 32
        f1p[:, k * 1400:(k + 1) * 1400] = arr.reshape(128, 1400)
    return C, TH, X8, f1p, slots


# -------------------------------------------------- activation-table surgery
def _fuse_act_tables(nc):
    """Rewrite the compiler-inserted per-function activation table loads into
    one preamble load of a set containing every function the kernel uses.

    The greedy chooser picks the first set containing each function (exp ->
    set 0, ln -> set 5) and reloads on every switch, putting a ~1.5us
    ACT_TABLE_LOAD between Exp and Ln on the critical path.  A single set
    (natural_log_exp_and_others) contains exp, ln, relu, identity and copy,
    so one load before the first activation covers the whole kernel."""
    from concourse.hw_specs import get_activation_tables
    tables = list(get_activation_tables(nc.m.arch).items())
    used = set()
    loads = []
    for b in nc.main_func.blocks:
        for i in b.instructions:
            if isinstance(i, mybir.InstActivation):
                used.add(i.func)
            elif isinstance(i, mybir.InstLoadActFuncSet):
                loads.append((b, i))
    if len(loads) <= 1:
        return
    pick = None
    for idx, (_, fns) in enumerate(tables):
        if used <= fns:
            pick = idx
            break
    if pick is None:
        return
    first = True
    for b, i in loads:
        if first:
            i.act_func_set_id = pick
            first = False
            continue
        si = i.sync_info
        if si is not None and (si.on_wait or si.on_update):
            continue  # carries sync - leave it (redundant but harmless)
        b.instructions.remove(i)


# ---------------------------------------------------------- device program
def _build(steps, fc_live, ncols32, ncols16, ncols8, nfc):
    nc = bacc.Bacc("TRN2", target_bir_lowering=False)
    consts_d = nc.dram_tensor("consts", [128, ncols32], F32, kind="ExternalInput")
    mainh_d = nc.dram_tensor("mainh", [128, ncols16], F16, kind="ExternalInput")
    xs_d = nc.dram_tensor("xs", [128, ncols8], F8, kind="ExternalInput")
    f1w_d = nc.dram_tensor("f1w", [128, 1400 * nfc], F16, kind="ExternalInput")
    out_d = nc.dram_tensor("out", [B, N_CLS], F32, kind="ExternalOutput")

    feeds_conv = set()
    for v, chans in steps:
        for skey, _ in chans:
            if skey != 'x':
                feeds_conv.add(skey)
    fc_srcs = [s for _, s in fc_live]
    SL = _SLOTS
    H2 = FC_HID - 128  # 72

    with tile.TileContext(nc) as tc:
        with (
            tc.tile_pool(name="persist", bufs=1) as pool,
            tc.tile_pool(name="cpsum", bufs=2, space="PSUM") as cpp,
            tc.tile_pool(name="fpsum", bufs=1, space="PSUM") as fpp,
        ):
            consts = pool.tile([128, ncols32], F32, tag="consts")
            mainh = pool.tile([128, ncols16], F16, tag="mainh")
            xs = pool.tile([128, ncols8], F8, tag="xs")
            f1w = pool.tile([128, 1400 * nfc], F16, tag="f1w")

            # All input DMAs on the sync queue (one hardware ring keeps the
            # semaphore-reset epilogue small).  No PE warmup: the HAM clock
            # gate on this part never un-throttles (verified over 5.7us of
            # continuous matmuls), so every matmul runs at 1.2 GHz and
            # warmup would only delay the DMAs.  xs (fp8, image + conv0
            # toeplitz) goes first - it alone gates conv0.
            nc.sync.dma_start(xs[:], xs_d[:])
            nc.sync.dma_start(consts[:], consts_d[:])
            nc.sync.dma_start(mainh[:], mainh_d[:])
            nc.sync.dma_start(f1w[:], f1w_d[:])

            # Trigger the single activation-table load (rewritten to the
            # exp+ln set by _fuse_act_tables) off the critical path.
            swu = pool.tile([1, 2], F32, tag="swu")
            nc.vector.memset(swu[:], 1.0)
            nc.scalar.activation(swu[:, 0:1], swu[:, 0:1], AF.Exp)

            # Activation stacks (fp16, zero borders double as y-padding) and
            # the fc readout stack; h2e's extra row of ones folds fc2_b in.
            stacks = {}
            for v in sorted(feeds_conv):
                a = pool.tile([128, 544], F16, name=f"st_{v}", tag=f"st_{v}")
                nc.vector.memset(a[:], 0.0)
                stacks[v] = a
            fcstacks = {}
            for sv in sorted(set(fc_srcs)):
                t = pool.tile([128, 112 * 1], F16, name=f"fcst_{sv}",
                              tag=f"fcst_{sv}")
                nc.vector.memset(t[:], 0.0)
                fcstacks[sv] = t
            h1 = pool.tile([128, B], F16, tag="h1")
            h2e = pool.tile([H2 + 1, B], F16, tag="h2e")
            nc.vector.memset(h2e[:], 1.0)

            def movA(key):
                return xs[:, SL['xs'] + 32:SL['xs'] + 480] if key == 'x' \
                    else stacks[key][:, 32:480]

            def movB(key):
                return xs[0:32, SL['xs'] + 96:SL['xs'] + 544] if key == 'x' \
                    else stacks[key][0:32, 96:544]

            def statA(v, j, skey):
                if skey == 'x':
                    a8 = SL[('xtoepA', v, j)]
                    return xs[:, a8:a8 + HW]
                a0 = SL[('toepA', v, j)]
                return mainh[:, a0:a0 + HW]

            def statB(v, j, skey):
                if skey == 'x':
                    b8 = SL[('xtoepB', v, j)]
                    return xs[0:32, b8:b8 + HW]
                b0 = SL[('toepB', v, j)]
                return mainh[:32, b0:b0 + HW]

            # --- conv chain ---
            # Tile hazard tracking is partition-blind (column-overlap on the
            # same tile serializes), so the stack fanout is one ACT bias+relu
            # into group 2 followed by three cheap 16-bit DVE copies; fc-only
            # nodes pipeline quarter matmul pairs against the quarter writes.
            for v, chans in steps:
                cb0 = SL[('cb', v)]
                bias = consts[:HW, cb0:cb0 + 1]
                nch = len(chans)
                fc_only = v in fcstacks and v not in feeds_conv

                if fc_only:
                    fst = fcstacks[v]
                    for g in range(4):
                        psq = cpp.tile([HW, 112], F32, tag="convq", bufs=2,
                                       name=f"psq{v}_{g}")
                        for j, (skey, ch) in enumerate(chans):
                            cA = g * 112
                            nc.tensor.matmul(
                                psq[:], statA(v, j, skey),
                                movA(skey)[:, cA:cA + 112],
                                start=(j == 0), stop=False)
                            nc.tensor.matmul(
                                psq[:], statB(v, j, skey),
                                movB(skey)[:, cA:cA + 112],
                                start=False, stop=(j == nch - 1))
                        dst = fst[g * 32:g * 32 + HW, :]
                        if g % 2 == 0:
                            nc.scalar.activation(dst, psq[:], AF.Relu,
                                                 bias=bias, scale=1.0)
                        else:
                            nc.vector.tensor_scalar(dst, psq[:], bias, 0.0,
                                                    ALU.add, ALU.max)
                    continue

                ps = cpp.tile([HW, 448], F32, tag="convps", name=f"ps{v}")
                for j, (skey, ch) in enumerate(chans):
                    nc.tensor.matmul(ps[:], statA(v, j, skey), movA(skey),
                                     start=(j == 0), stop=False)
                    nc.tensor.matmul(ps[:], statB(v, j, skey), movB(skey),
                                     start=False, stop=(j == nch - 1))

                if v in feeds_conv:
                    st = stacks[v]
                    g2 = st[64:64 + HW, 32:480]
                    nc.scalar.activation(g2, ps[:], AF.Relu, bias=bias,
                                         scale=1.0)
                    for g in (0, 1, 3):
                        c0 = (4 - g) * 16
                        nc.vector.tensor_copy(
                            st[g * 32:g * 32 + HW, c0:c0 + 448], g2)
                if v in fcstacks:
                    fst = fcstacks[v]
                    for g in range(4):
                        dst = fst[g * 32:g * 32 + HW, :]
                        srcp = ps[:, g * 112:(g + 1) * 112]
                        if g % 2 == 0:
                            nc.scalar.activation(dst, srcp, AF.Relu,
                                                 bias=bias, scale=1.0)
                        else:
                            nc.vector.tensor_scalar(dst, srcp, bias, 0.0,
                                                    ALU.add, ALU.max)

            # --- fc1 transposed: hidden units on partitions ---
            p1a = fpp.tile([128, B], F32, tag="p1a", bufs=1)
            p1b = fpp.tile([H2, B], F32, tag="p1b", bufs=1)
            nmm = nfc * 7
            for k in range(nfc):
                fst = fcstacks[fc_live[k][1]]
                for sj in range(7):
                    i = k * 7 + sj
                    mov = fst[:, sj * 16:(sj + 1) * 16]
                    c0 = i * 200
                    nc.tensor.matmul(p1a[:], f1w[:, c0:c0 + 128], mov,
                                     start=(i == 0), stop=(i == nmm - 1))
                    nc.tensor.matmul(p1b[:], f1w[:, c0 + 128:c0 + 200], mov,
                                     start=(i == 0), stop=(i == nmm - 1))
            nc.scalar.activation(h1[:], p1a[:], AF.Relu,
                                 bias=consts[:128, SL['fc1bA']:SL['fc1bA'] + 1],
                                 scale=1.0)
            nc.vector.tensor_scalar(h2e[0:H2, :], p1b[:],
                                    consts[:H2, SL['fc1bB']:SL['fc1bB'] + 1],
                                    0.0, ALU.add, ALU.max)

            # --- fc2 + log_softmax (no max-subtraction; logits are small) ---
            pst = fpp.tile([B, N_CLS], F32, tag="pst", bufs=1)
            nc.tensor.matmul(pst[:], h1[:], mainh[:, SL['w2A']:SL['w2A'] + N_CLS],
                             start=True, stop=False)
            nc.tensor.matmul(pst[:], h2e[:],
                             mainh[:H2 + 1, SL['w2B']:SL['w2B'] + N_CLS],
                             start=False, stop=True)
            ex = pool.tile([B, N_CLS], F32, tag="ex")
            sm = pool.tile([B, 1], F32, tag="sm")
            nc.scalar.activation(ex[:], pst[:], AF.Exp, bias=0.0, scale=1.0,
                                 accum_out=sm[:])
            lse = pool.tile([B, 1], F32, tag="lse")
            nc.scalar.activation(lse[:], sm[:], AF.Ln, bias=0.0, scale=1.0)
            res = pool.tile([B, N_CLS], F32, tag="res")
            nc.vector.tensor_scalar(res[:], pst[:], lse[:], None, ALU.subtract)
            nc.sync.dma_start(out_d[:], res[:])
    nc.compile()
    _fuse_act_tables(nc)
    return nc


_SLOTS = None
_PROG_CACHE = {}


def kernel(x, src, tgt, conv_w, conv_b, fc1_w, fc1_b, fc2_w, fc2_b):
    global _SLOTS, LAST_RESULT
    x = np.asarray(x, np.float32)
    src = np.asarray(src, np.int32)
    tgt = np.asarray(tgt, np.int32)
    conv_w = np.asarray(conv_w, np.float32)
    conv_b = np.asarray(conv_b, np.float32)
    fc1_w = np.asarray(fc1_w, np.float32)
    fc1_b = np.asarray(fc1_b, np.float32)
    fc2_w = np.asarray(fc2_w, np.float32)
    fc2_b = np.asarray(fc2_b, np.float32)

    steps, fc_live = _cone(src, tgt)
    C, TH, X8, f1p, slots = _pack(steps, fc_live, conv_w, conv_b,
                                  fc1_w, fc1_b, fc2_w, fc2_b)
    _SLOTS = slots
    import ml_dtypes
    E4M3 = ml_dtypes.float8_e4m3

    key = (tuple((v, tuple(ch)) for v, ch in steps), tuple(fc_live),
           C.shape[1], TH.shape[1], X8.shape[1])
    if key not in _PROG_CACHE:
        _PROG_CACHE[key] = _build(steps, fc_live, C.shape[1], TH.shape[1],
                                  X8.shape[1], len(fc_live))
    nc = _PROG_CACHE[key]

    xs_all = x[:, 0]  # [128, 28, 28]
    in_maps = []
    for c in range(N_CORES):
        x8 = X8.copy()
        x8[:, slots['xs']:slots['xs'] + 544] = _xstack(xs_all[c * B:(c + 1) * B])
        in_maps.append({"consts": C, "mainh": TH, "f1w": f1p,
                        "xs": x8.astype(E4M3)})

    LAST_RESULT = run_bass_kernel_spmd(nc, in_maps, list(range(N_CORES)))
    out = np.concatenate([r["out"] for r in LAST_RESULT.results], axis=0)
    return out.astype(np.float32)


# revision 14
# speedup vs baseline: 1.1345x; 1.0873x over previous
"""Trainium2 Bass kernel for nn_Net_21852793602541 (gnn_message_passing).

The reference net's output depends only on a tiny dependency cone of the
message-passing graph: the final hidden layer reads the wave-2 snapshot of
neuron activations, so only neurons feeding neuron 255 through channels whose
source was already processed matter.  For the fixed graph that is a 3-conv
chain (x -> n0 -> n172 -> n215), one 784->200 FC block, a 200->10 FC and
log_softmax.  The cone is recomputed at runtime from the src/tgt inputs.

Per-core mapping (data-parallel over batch, 16 images/core on 8 cores):
  * 5x5 conv == one PE accumulation group: contraction K = (dy, slot-row)
    with a banded-Toeplitz stationary.  Activations live in a single
    [128, 544] stack whose free axis is (ypad, b): the four dy<=3 groups are
    y-shifted replicas on partition groups, and the dy=4 tap is realised as a
    column-offset read of group 0 - no fifth replica, no 32-partition tail.
  * Tile hazard tracking is partition-blind and full-tile for writes, so the
    stack fanout is one ACT bias+relu from PSUM into group 2 followed by
    three cheap 16-bit DVE copies; fc-only nodes pipeline quarter matmul
    pairs against alternating ACT/DVE quarter writes.
  * The input image and conv0's toeplitz ride a single fp8e4 tensor (half
    the DMA bytes of fp16; rel err ~5e-4), issued first on the sync queue -
    it alone gates conv0.  Later stacks are fp16.
  * fc1 is computed transposed (hidden on partitions) via 14 small matmuls so
    no PSUM->SBUF copy or PE transpose is needed; fc2 takes the hidden tiles
    as stationary with fc2_b folded in through an all-ones row.
  * log_softmax drops the max-subtraction (logits are small), uses Exp with
    fused accumulation and Ln; a post-compile pass rewrites all activation
    table loads into a single preamble load of the set containing both exp
    and ln, so no table switch lands on the critical path.
  * No PE warmup: the HAM clock gate on this part never un-throttles
    (verified over 5.7us of continuous matmuls), so everything runs at
    1.2 GHz and dummy matmuls would only delay the input DMAs.
"""

import numpy as np

import concourse.bass as bass
import concourse.tile as tile
from concourse import bacc, mybir
from concourse.bass_utils import run_bass_kernel_spmd

# The axon NTFF profile hook normally lives in antenv.axon_hooks, which this
# image lacks.  Shim it from the boot module's ctypes implementation so
# BASS_TRACE=1 profiling works; degrade silently if unavailable.
try:
    import antenv.axon_hooks  # noqa: F401
except ImportError:
    try:
        import sys as _sys
        import types as _types

        from trn_agent_boot.trn_boot import _ntff_profile_via_ctypes

        _hook = _ntff_profile_via_ctypes('/opt/axon/libaxon_pjrt.so')
        _mod = _types.ModuleType('antenv.axon_hooks')
        _mod.get_axon_ntff_profile_hook = lambda: _hook
        _mod.set_axon_ntff_profile_hook = lambda h: None
        _sys.modules['antenv.axon_hooks'] = _mod
    except Exception:
        pass

F32 = mybir.dt.float32
F16 = mybir.dt.float16
F8 = mybir.dt.float8e4
AF = mybir.ActivationFunctionType
ALU = mybir.AluOpType
N_NEURONS = 256
N_CORES = 8
B_TOTAL = 128
B = B_TOTAL // N_CORES  # 16 images per core
HW = 28
FC_HID = 200
N_CLS = 10

LAST_RESULT = None  # BassKernelResults of the most recent run (for profiling)


# ---------------------------------------------------------------- schedule
def _schedule(src, tgt):
    n = N_NEURONS
    in_lists = [src[np.where(tgt == i)[0]].astype(np.int64).tolist() for i in range(n)]
    waves = []
    processed = np.zeros(n, bool)
    frontier = [0]
    while True:
        waves.append(list(frontier))
        processed[frontier] = True
        if processed[n - 1]:
            break
        nxt = set()
        for v in frontier:
            for m in tgt[src == v]:
                if not processed[m]:
                    nxt.add(int(m))
        frontier = sorted(nxt)
        assert frontier, "last neuron unreachable"
    return in_lists, waves


def _cone(src, tgt):
    """Returns (steps, fc_live).

    steps: ordered list of (node, [(srckey, channel), ...]) where srckey is
      'x' for the image input or an int neuron id computed in an earlier step.
    fc_live: [(channel_of_255, src_node), ...] live channels of the readout.
    """
    n = N_NEURONS
    in_lists, waves = _schedule(src, tgt)
    wave_of = {}
    for wi, w in enumerate(waves):
        for v in w:
            if v not in wave_of:
                wave_of[v] = wi
    BIG = 1 << 30
    w255 = wave_of[n - 1]
    fc_live = [(c, int(s)) for c, s in enumerate(in_lists[n - 1])
               if wave_of.get(int(s), BIG) < w255]

    live = {}
    stack = [s for _, s in fc_live]
    seen = set()
    while stack:
        v = stack.pop()
        if v in seen:
            continue
        seen.add(v)
        if v == 0:
            live[0] = [('x', 0)]
            continue
        chans = [(int(s), c) for c, s in enumerate(in_lists[v])
                 if wave_of.get(int(s), BIG) < wave_of[v]]
        assert chans, f"cone node {v} has no live channels"
        live[v] = [(s, c) for s, c in chans]
        stack += [s for s, _ in chans]

    steps = sorted(live.items(), key=lambda kv: wave_of[kv[0]])
    return steps, fc_live


# ---------------------------------------------------------- host-side packing
def _toeplitz(w):
    """w [5,5] -> [160, 28] banded matrix over K=(dy, slot-row).

    Slot row r of each 32-row group holds padded-image column (r+2) mod 32,
    so the activation value at x lands at row x (32-aligned writes; wrapped
    rows 28..31 hold the zero x-padding)."""
    T = np.zeros((160, HW), np.float32)
    for dy in range(5):
        for dx in range(5):
            for xc in range(HW):
                T[dy * 32 + (xc + dx - 2) % 32, xc] = w[dy, dx]
    return T


def _xstack(xb):
    """xb [B,28,28] -> [128, 544] stack (fp32; caller casts), free axis = (ypad, b).

    Group g (dy=g<=3), slot row r, column p*16+b holds
    xpad[b, p+g-2, (r+2) % 32] (zero when the y index is out of range).
    The dy=4 tap is read from group 0 at a +96-column offset on device
    (columns [512, 544) stay zero - the overhang rows are y-padding)."""
    xpad = np.zeros((B, 32, 32), np.float32)
    xpad[:, 2:30, 2:30] = xb
    st = np.zeros((4, 32, 34, B), np.float32)  # g, slot-x, ypad, b
    for g in range(4):
        lo, hi = max(0, 2 - g), min(32, 34 - g)
        st[g, :, lo:hi, :] = xpad[:, lo + g - 2:hi + g - 2, :].transpose(2, 1, 0)
    st = np.roll(st, -2, axis=1)  # slot row r holds padded col (r+2) % 32
    return st.reshape(128, 34 * B)


class _Layout:
    def __init__(self):
        self.n = 0

    def alloc(self, w):
        c0 = self.n
        self.n += w
        return c0


def _pack(steps, fc_live, conv_w, conv_b, fc1_w, fc1_b, fc2_w, fc2_b):
    """Builds consts (f32), mainh head block (fp16), f1w (fp16)."""
    slots = {}
    lay32 = _Layout()
    lay16 = _Layout()
    for v, chans in steps:
        for j, _ in enumerate(chans):
            slots[('toepA', v, j)] = lay16.alloc(HW)
            slots[('toepB', v, j)] = lay16.alloc(HW)
        slots[('cb', v)] = lay32.alloc(1)
    slots['fc1bA'] = lay32.alloc(1)
    slots['fc1bB'] = lay32.alloc(1)
    slots['w2A'] = lay16.alloc(N_CLS)
    slots['w2B'] = lay16.alloc(N_CLS)
    head_cols = lay16.n

    # x-sourced channels run entirely in fp8e4 (input image + conv0 toeplitz
    # share the xs tensor - one DMA, half the bytes, rel err ~4e-4)
    lay8 = _Layout()
    slots['xs'] = lay8.alloc(544)
    for v, chans in steps:
        for j, (skey, ch) in enumerate(chans):
            if skey == 'x':
                slots[('xtoepA', v, j)] = lay8.alloc(HW)
                slots[('xtoepB', v, j)] = lay8.alloc(HW)
    xs_cols = lay8.n

    C = np.zeros((128, lay32.n), np.float32)
    TH = np.zeros((128, head_cols), np.float16)
    X8 = np.zeros((128, xs_cols), np.float32)  # cast to fp8 by caller
    for v, chans in steps:
        for j, (skey, ch) in enumerate(chans):
            T = _toeplitz(conv_w[v, 0, ch])
            TH[:, slots[('toepA', v, j)]:slots[('toepA', v, j)] + HW] = T[:128]
            TH[:32, slots[('toepB', v, j)]:slots[('toepB', v, j)] + HW] = T[128:]
            if skey == 'x':
                a8 = slots[('xtoepA', v, j)]
                b8 = slots[('xtoepB', v, j)]
                X8[:, a8:a8 + HW] = T[:128]
                X8[:32, b8:b8 + HW] = T[128:]
        C[:HW, slots[('cb', v)]] = conv_b[v]
    C[:128, slots['fc1bA']] = fc1_b[:128]
    C[:FC_HID - 128, slots['fc1bB']] = fc1_b[128:]
    w2t = fc2_w.T  # [200, 10]
    TH[:128, slots['w2A']:slots['w2A'] + N_CLS] = w2t[:128]
    TH[:FC_HID - 128, slots['w2B']:slots['w2B'] + N_CLS] = w2t[128:]
    TH[FC_HID - 128, slots['w2B']:slots['w2B'] + N_CLS] = fc2_b  # ones-row bias

    # fc1 transposed: stationary chunks [128, 200] per (live channel, ysub).
    # Partition = yg*32 + x, column (k*7+sj)*200 + j = fc1_w[j, pixel].
    n_fc = len(fc_live)
    f1p = np.zeros((128, 1400 * n_fc), np.float16)
    for k, (c, s) in enumerate(fc_live):
        blk = fc1_w[:, c * 784:(c + 1) * 784].reshape(FC_HID, 4, 7, HW)  # j,g,sj,x
        arr = blk.transpose(1, 3, 2, 0)  # g, x, sj, j
        arr = np.pad(arr, ((0, 0), (0, 4), (0, 0), (0, 0)))  # x -> 32
        f1p[:, k * 1400:(k + 1) * 1400] = arr.reshape(128, 1400)
    return C, TH, X8, f1p, slots


# -------------------------------------------------- activation-table surgery
def _fuse_act_tables(nc):
    """Rewrite the compiler-inserted per-function activation table loads into
    one preamble load of a set containing every function the kernel uses.

    The greedy chooser picks the first set containing each function (exp ->
    set 0, ln -> set 5) and reloads on every switch, putting a ~1.5us
    ACT_TABLE_LOAD between Exp and Ln on the critical path.  A single set
    (natural_log_exp_and_others) contains exp, ln, relu, identity and copy,
    so one load before the first activation covers the whole kernel."""
    from concourse.hw_specs import get_activation_tables
    tables = list(get_activation_tables(nc.m.arch).items())
    used = set()
    loads = []
    for b in nc.main_func.blocks:
        for i in b.instructions:
            if isinstance(i, mybir.InstActivation):
                used.add(i.func)
            elif isinstance(i, mybir.InstLoadActFuncSet):
                loads.append((b, i))
    if len(loads) <= 1:
        return
    pick = None
    for idx, (_, fns) in enumerate(tables):
        if used <= fns:
            pick = idx
            break
    if pick is None:
        return
    first = True
    for b, i in loads:
        if first:
            i.act_func_set_id = pick
            first = False
            continue
        si = i.sync_info
        if si is not None and (si.on_wait or si.on_update):
            continue  # carries sync - leave it (redundant but harmless)
        b.instructions.remove(i)


# ---------------------------------------------------------- device program
def _build(steps, fc_live, ncols32, ncols16, ncols8, nfc):
    nc = bacc.Bacc("TRN2", target_bir_lowering=False)
    consts_d = nc.dram_tensor("consts", [128, ncols32], F32, kind="ExternalInput")
    mainh_d = nc.dram_tensor("mainh", [128, ncols16], F16, kind="ExternalInput")
    xs_d = nc.dram_tensor("xs", [128, ncols8], F8, kind="ExternalInput")
    f1w_d = nc.dram_tensor("f1w", [128, 1400 * nfc], F16, kind="ExternalInput")
    out_d = nc.dram_tensor("out", [B, N_CLS], F32, kind="ExternalOutput")

    feeds_conv = set()
    for v, chans in steps:
        for skey, _ in chans:
            if skey != 'x':
                feeds_conv.add(skey)
    fc_srcs = [s for _, s in fc_live]
    SL = _SLOTS
    H2 = FC_HID - 128  # 72

    with tile.TileContext(nc) as tc:
        with (
            tc.tile_pool(name="persist", bufs=1) as pool,
            tc.tile_pool(name="cpsum", bufs=2, space="PSUM") as cpp,
            tc.tile_pool(name="fpsum", bufs=1, space="PSUM") as fpp,
        ):
            consts = pool.tile([128, ncols32], F32, tag="consts")
            mainh = pool.tile([128, ncols16], F16, tag="mainh")
            xs = pool.tile([128, ncols8], F8, tag="xs")
            f1w = pool.tile([128, 1400 * nfc], F16, tag="f1w")

            # xs (fp8, image + conv0 toeplitz) alone gates conv0, so it gets
            # the sync ring to itself: queueing more DMAs behind it delays
            # its completion semaphore by ~2us (one straggler stripe engine
            # serves the next DMA's descriptors first).  Everything else
            # rides the gpsimd ring.  No PE warmup: the HAM clock gate on
            # this part never un-throttles (verified over 5.7us of
            # continuous matmuls), so every matmul runs at 1.2 GHz and
            # warmup would only delay the DMAs.
            nc.sync.dma_start(xs[:], xs_d[:])
            nc.gpsimd.dma_start(consts[:], consts_d[:])
            nc.gpsimd.dma_start(mainh[:], mainh_d[:])
            nc.gpsimd.dma_start(f1w[:], f1w_d[:])

            # Trigger the single activation-table load (rewritten to the
            # exp+ln set by _fuse_act_tables) off the critical path.
            swu = pool.tile([1, 2], F32, tag="swu")
            nc.vector.memset(swu[:], 1.0)
            nc.scalar.activation(swu[:, 0:1], swu[:, 0:1], AF.Exp)

            # Activation stacks (fp16, zero borders double as y-padding) and
            # the fc readout stack; h2e's extra row of ones folds fc2_b in.
            stacks = {}
            for v in sorted(feeds_conv):
                a = pool.tile([128, 544], F16, name=f"st_{v}", tag=f"st_{v}")
                nc.vector.memset(a[:], 0.0)
                stacks[v] = a
            fcstacks = {}
            for sv in sorted(set(fc_srcs)):
                t = pool.tile([128, 112 * 1], F16, name=f"fcst_{sv}",
                              tag=f"fcst_{sv}")
                nc.vector.memset(t[:], 0.0)
                fcstacks[sv] = t
            h1 = pool.tile([128, B], F16, tag="h1")
            h2e = pool.tile([H2 + 1, B], F16, tag="h2e")
            nc.vector.memset(h2e[:], 1.0)

            def movA(key):
                return xs[:, SL['xs'] + 32:SL['xs'] + 480] if key == 'x' \
                    else stacks[key][:, 32:480]

            def movB(key):
                return xs[0:32, SL['xs'] + 96:SL['xs'] + 544] if key == 'x' \
                    else stacks[key][0:32, 96:544]

            def statA(v, j, skey):
                if skey == 'x':
                    a8 = SL[('xtoepA', v, j)]
                    return xs[:, a8:a8 + HW]
                a0 = SL[('toepA', v, j)]
                return mainh[:, a0:a0 + HW]

            def statB(v, j, skey):
                if skey == 'x':
                    b8 = SL[('xtoepB', v, j)]
                    return xs[0:32, b8:b8 + HW]
                b0 = SL[('toepB', v, j)]
                return mainh[:32, b0:b0 + HW]

            # --- conv chain ---
            # Tile hazard tracking is partition-blind (column-overlap on the
            # same tile serializes), so the stack fanout is one ACT bias+relu
            # into group 2 followed by three cheap 16-bit DVE copies; fc-only
            # nodes pipeline quarter matmul pairs against the quarter writes.
            for v, chans in steps:
                cb0 = SL[('cb', v)]
                bias = consts[:HW, cb0:cb0 + 1]
                nch = len(chans)
                fc_only = v in fcstacks and v not in feeds_conv

                if fc_only:
                    fst = fcstacks[v]
                    for g in range(4):
                        psq = cpp.tile([HW, 112], F32, tag="convq", bufs=2,
                                       name=f"psq{v}_{g}")
                        for j, (skey, ch) in enumerate(chans):
                            cA = g * 112
                            nc.tensor.matmul(
                                psq[:], statA(v, j, skey),
                                movA(skey)[:, cA:cA + 112],
                                start=(j == 0), stop=False)
                            nc.tensor.matmul(
                                psq[:], statB(v, j, skey),
                                movB(skey)[:, cA:cA + 112],
                                start=False, stop=(j == nch - 1))
                        dst = fst[g * 32:g * 32 + HW, :]
                        if g % 2 == 0:
                            nc.scalar.activation(dst, psq[:], AF.Relu,
                                                 bias=bias, scale=1.0)
                        else:
                            nc.vector.tensor_scalar(dst, psq[:], bias, 0.0,
                                                    ALU.add, ALU.max)
                    continue

                ps = cpp.tile([HW, 448], F32, tag="convps", name=f"ps{v}")
                for j, (skey, ch) in enumerate(chans):
                    nc.tensor.matmul(ps[:], statA(v, j, skey), movA(skey),
                                     start=(j == 0), stop=False)
                    nc.tensor.matmul(ps[:], statB(v, j, skey), movB(skey),
                                     start=False, stop=(j == nch - 1))

                if v in feeds_conv:
                    st = stacks[v]
                    g2 = st[64:64 + HW, 32:480]
                    nc.scalar.activation(g2, ps[:], AF.Relu, bias=bias,
                                         scale=1.0)
                    for g in (0, 1, 3):
                        c0 = (4 - g) * 16
                        nc.vector.tensor_copy(
                            st[g * 32:g * 32 + HW, c0:c0 + 448], g2)
                if v in fcstacks:
                    fst = fcstacks[v]
                    for g in range(4):
                        dst = fst[g * 32:g * 32 + HW, :]
                        srcp = ps[:, g * 112:(g + 1) * 112]
                        if g % 2 == 0:
                            nc.scalar.activation(dst, srcp, AF.Relu,
                                                 bias=bias, scale=1.0)
                        else:
                            nc.vector.tensor_scalar(dst, srcp, bias, 0.0,
                                                    ALU.add, ALU.max)

            # --- fc1 transposed: hidden units on partitions ---
            p1a = fpp.tile([128, B], F32, tag="p1a", bufs=1)
            p1b = fpp.tile([H2, B], F32, tag="p1b", bufs=1)
            nmm = nfc * 7
            for k in range(nfc):
                fst = fcstacks[fc_live[k][1]]
                for sj in range(7):
                    i = k * 7 + sj
                    mov = fst[:, sj * 16:(sj + 1) * 16]
                    c0 = i * 200
                    nc.tensor.matmul(p1a[:], f1w[:, c0:c0 + 128], mov,
                                     start=(i == 0), stop=(i == nmm - 1))
                    nc.tensor.matmul(p1b[:], f1w[:, c0 + 128:c0 + 200], mov,
                                     start=(i == 0), stop=(i == nmm - 1))
            nc.scalar.activation(h1[:], p1a[:], AF.Relu,
                                 bias=consts[:128, SL['fc1bA']:SL['fc1bA'] + 1],
                                 scale=1.0)
            nc.vector.tensor_scalar(h2e[0:H2, :], p1b[:],
                                    consts[:H2, SL['fc1bB']:SL['fc1bB'] + 1],
                                    0.0, ALU.add, ALU.max)

            # --- fc2 + log_softmax (no max-subtraction; logits are small) ---
            pst = fpp.tile([B, N_CLS], F32, tag="pst", bufs=1)
            nc.tensor.matmul(pst[:], h1[:], mainh[:, SL['w2A']:SL['w2A'] + N_CLS],
                             start=True, stop=False)
            nc.tensor.matmul(pst[:], h2e[:],
                             mainh[:H2 + 1, SL['w2B']:SL['w2B'] + N_CLS],
                             start=False, stop=True)
            ex = pool.tile([B, N_CLS], F32, tag="ex")
            sm = pool.tile([B, 1], F32, tag="sm")
            nc.scalar.activation(ex[:], pst[:], AF.Exp, bias=0.0, scale=1.0,
                                 accum_out=sm[:])
            lse = pool.tile([B, 1], F32, tag="lse")
            nc.scalar.activation(lse[:], sm[:], AF.Ln, bias=0.0, scale=1.0)
            res = pool.tile([B, N_CLS], F32, tag="res")
            nc.vector.tensor_scalar(res[:], pst[:], lse[:], None, ALU.subtract)
            nc.sync.dma_start(out_d[:], res[:])
    nc.compile()
    _fuse_act_tables(nc)
    return nc


_SLOTS = None
_PROG_CACHE = {}


def kernel(x, src, tgt, conv_w, conv_b, fc1_w, fc1_b, fc2_w, fc2_b):
    global _SLOTS, LAST_RESULT
    x = np.asarray(x, np.float32)
    src = np.asarray(src, np.int32)
    tgt = np.asarray(tgt, np.int32)
    conv_w = np.asarray(conv_w, np.float32)
    conv_b = np.asarray(conv_b, np.float32)
    fc1_w = np.asarray(fc1_w, np.float32)
    fc1_b = np.asarray(fc1_b, np.float32)
    fc2_w = np.asarray(fc2_w, np.float32)
    fc2_b = np.asarray(fc2_b, np.float32)

    steps, fc_live = _cone(src, tgt)
    C, TH, X8, f1p, slots = _pack(steps, fc_live, conv_w, conv_b,
                                  fc1_w, fc1_b, fc2_w, fc2_b)
    _SLOTS = slots
    import ml_dtypes
    E4M3 = ml_dtypes.float8_e4m3

    key = (tuple((v, tuple(ch)) for v, ch in steps), tuple(fc_live),
           C.shape[1], TH.shape[1], X8.shape[1])
    if key not in _PROG_CACHE:
        _PROG_CACHE[key] = _build(steps, fc_live, C.shape[1], TH.shape[1],
                                  X8.shape[1], len(fc_live))
    nc = _PROG_CACHE[key]

    xs_all = x[:, 0]  # [128, 28, 28]
    in_maps = []
    for c in range(N_CORES):
        x8 = X8.copy()
        x8[:, slots['xs']:slots['xs'] + 544] = _xstack(xs_all[c * B:(c + 1) * B])
        in_maps.append({"consts": C, "mainh": TH, "f1w": f1p,
                        "xs": x8.astype(E4M3)})

    LAST_RESULT = run_bass_kernel_spmd(nc, in_maps, list(range(N_CORES)))
    out = np.concatenate([r["out"] for r in LAST_RESULT.results], axis=0)
    return out.astype(np.float32)
